# revision 7
# baseline (speedup 1.0000x reference)
"""CosHead kernel for Trainium2 (8 NeuronCores, Bass/Tile).

out[c, h, w] = cos_sim(x[:, h, w], weights[c]) * scale[c] * 5.0

Sharding: spatial (H) split across the 8 cores — each core reads only its
1/8 slice of x and writes its 1/8 slice of the output (minimum possible
HBM traffic; the class-split in the sharding hint would replicate all of
x onto every core).

v2: 16-bit I/O. x is cast to fp16 on the host and the output is written
fp16 and upcast on the host, halving HBM traffic vs the fp32 baseline:
4.2 MB in + 4.2 MB out per core -> ~23.4 us at ~358 GB/s/core (vs ~47 us
for fp32 I/O, which is what the previous version measured). fp16
quantization error lands ~1e-4 absmax-relative, far below the 2e-2 gate.

Per-core device pipeline (npix = 8192 pixels, D = 256, C = 256):
  - DMA in x as two partition chunks [128, stage] fp16 (D on partitions).
  - ACT: q0 = x0^2 (fp16); DVE: q1 = x1*x1 (fp16, 2x-packed mode).
  - PE:  pn = ones[128,128].T @ q (accumulated over the two D chunks)
         -> PSUM strip [128, stage]; rows broadcast per-pixel sum-sq.
  - ACT: inv = Rsqrt(pn) — one op replaces the old Sqrt+DVE-reciprocal.
  - PE:  p = wfoldT.T @ x (fp16 matmuls accumulated over D chunks) into
         [128, 1024] PSUM strips, where wfoldT[d, c] folds
         5 * scale[c] / max(||w_c||, eps) on the host (O(C*D) work).
  - DVE: o = p * inv (PSUM fp32 -> SBUF fp16), 1024 wide.
  - DMA out per stage from the scalar engine's ring (keeps output DMAs
    off the sync ring that issues input DMAs).

PSUM: ppn strip [128, stage=2048] = 4 banks + pp 2 bufs x [128,1024]
= 4 banks -> exactly 8.

Engine budgets/core: DMA 23.4us (roof), PE 20.5us, ACT ~15us, DVE ~24us.
"""

import numpy as np
from contextlib import ExitStack

import concourse.bacc as bacc
import concourse.tile as tile
from concourse import mybir
from concourse.bass_utils import run_bass_kernel_spmd

N_CORES = 8
C = 256           # n_classes
D = 256           # latent
H = 256
W = 256
HL = H // N_CORES # 32 rows of H per core
NPIX = HL * W     # 8192 pixels per core
EPS = 1e-8
RANGE_EXTENDER = 5.0

STAGE = 2048      # pixels per pipeline stage
PT = 512          # pixels per matmul (one fp32 PSUM bank)
PW = 1024         # pixels per main-matmul PSUM strip / DVE mul

F32 = mybir.dt.float32
F16 = mybir.dt.float16
BF16 = mybir.dt.bfloat16

_CACHE = {}


def build(repeat=1, staggered=False, stage=STAGE, pw=PW, bufs=3,
          q1_engine="split", dma_only=False):
    """Build + compile the SPMD per-core program. repeat>1 wraps the whole
    pipeline in a hardware loop (for slope-method timing)."""
    nc = bacc.Bacc("TRN2", target_bir_lowering=False, debug=False)
    x_t = nc.dram_tensor("x", [2, 128, NPIX], F16, kind="ExternalInput")
    w_t = nc.dram_tensor("wt", [2, 128, C], F16, kind="ExternalInput")
    o_t = nc.dram_tensor("out", [2, 128, NPIX], F16, kind="ExternalOutput")
    x_d, w_d, o_d = x_t.ap(), w_t.ap(), o_t.ap()

    with ExitStack() as ctx:
        tc = ctx.enter_context(tile.TileContext(nc))
        consts = ctx.enter_context(tc.tile_pool(name="consts", bufs=1))
        xp = ctx.enter_context(tc.tile_pool(name="xp", bufs=bufs))
        qp = ctx.enter_context(tc.tile_pool(name="qp", bufs=2))
        vp = ctx.enter_context(tc.tile_pool(name="vp", bufs=2))
        op = ctx.enter_context(tc.tile_pool(name="op", bufs=2))
        # pn strip (4 banks) + p0 + p1 (2 banks each) = exactly 8 PSUM banks
        pp = ctx.enter_context(tc.tile_pool(name="pp", bufs=1, space="PSUM"))

        w0 = consts.tile([128, C], F16)
        nc.sync.dma_start(w0[:], w_d[0])
        w1 = consts.tile([128, C], F16)
        nc.sync.dma_start(w1[:], w_d[1])
        ones = consts.tile([128, 128], F16)
        nc.vector.memset(ones[:], 1.0)

        def body():
            if dma_only:
                for s in range(NPIX // stage):
                    c0 = s * stage
                    x0 = xp.tile([128, stage], F16, tag="x0")
                    nc.sync.dma_start(x0[:], x_d[0, :, c0:c0 + stage])
                    x1 = xp.tile([128, stage], F16, tag="x1")
                    nc.sync.dma_start(x1[:], x_d[1, :, c0:c0 + stage])
                    o0 = op.tile([128, stage], F16, tag="o0")
                    nc.vector.tensor_copy(o0[:, 0:1], x0[:, 0:1])
                    o1 = op.tile([128, stage], F16, tag="o1")
                    nc.vector.tensor_copy(o1[:, 0:1], x1[:, 0:1])
                    nc.scalar.dma_start(o_d[0, :, c0:c0 + stage], o0[:])
                    nc.scalar.dma_start(o_d[1, :, c0:c0 + stage], o1[:])
                return
            for s in range(NPIX // stage):
                c0 = s * stage
                x0 = xp.tile([128, stage], F16, tag="x0")
                nc.sync.dma_start(x0[:], x_d[0, :, c0:c0 + stage])
                x1 = xp.tile([128, stage], F16, tag="x1")
                nc.sync.dma_start(x1[:], x_d[1, :, c0:c0 + stage])
                q0 = qp.tile([128, stage], F16, tag="q0")
                nc.scalar.activation(q0[:], x0[:],
                                     mybir.ActivationFunctionType.Square)
                # split the second square between ACT and DVE to balance
                q1 = qp.tile([128, stage], F16, tag="q1")
                if q1_engine == "vector":
                    nc.vector.tensor_mul(q1[:], x1[:], x1[:])
                elif q1_engine == "scalar":
                    nc.scalar.activation(q1[:], x1[:],
                                         mybir.ActivationFunctionType.Square)
                else:
                    hs = stage // 2
                    nc.scalar.activation(q1[:, 0:hs], x1[:, 0:hs],
                                         mybir.ActivationFunctionType.Square)
                    nc.vector.tensor_mul(q1[:, hs:stage], x1[:, hs:stage],
                                         x1[:, hs:stage])
                # per-pixel sum of squares, broadcast to all 128 partitions
                pn = pp.tile([128, stage], F32, tag="pn")
                for t in range(stage // PT):
                    sl = slice(t * PT, (t + 1) * PT)
                    nc.tensor.matmul(pn[:, sl], ones[:], q0[:, sl],
                                     start=True, stop=False)
                    nc.tensor.matmul(pn[:, sl], ones[:], q1[:, sl],
                                     start=False, stop=True)
                inv = vp.tile([128, stage], F32, tag="inv")
                nc.scalar.activation(inv[:], pn[:],
                                     mybir.ActivationFunctionType.Abs_reciprocal_sqrt)
                o0 = op.tile([128, stage], F16, tag="o0")
                o1 = op.tile([128, stage], F16, tag="o1")
                for u in range(stage // pw):
                    usl = slice(u * pw, (u + 1) * pw)
                    for h, oh in ((0, o0), (1, o1)):
                        hsl = slice(h * 128, (h + 1) * 128)
                        p = pp.tile([128, pw], F32, tag=f"p{h}")
                        for t in range(pw // PT):
                            tsl = slice(t * PT, (t + 1) * PT)
                            xsl = slice(u * pw + t * PT, u * pw + (t + 1) * PT)
                            nc.tensor.matmul(p[:, tsl], w0[:, hsl], x0[:, xsl],
                                             start=True, stop=False)
                            nc.tensor.matmul(p[:, tsl], w1[:, hsl], x1[:, xsl],
                                             start=False, stop=True)
                        nc.vector.tensor_mul(oh[:, usl], p[:], inv[:, usl])
                nc.scalar.dma_start(o_d[0, :, c0:c0 + stage], o0[:])
                nc.scalar.dma_start(o_d[1, :, c0:c0 + stage], o1[:])

        if repeat == 1:
            body()
        else:
            with tc.For_i(0, repeat, 1, staggered_reset=staggered):
                body()

    nc.compile()
    return nc


def _get_prog():
    key = "main"
    if key not in _CACHE:
        _CACHE[key] = build()
    return _CACHE[key]


def prep_inputs(x, weights, scale):
    """Host-side prep: shard x spatially (cast fp16), fold norm+scale into
    transposed fp16 weights. Returns in_maps for the 8 cores."""
    x = np.asarray(x, dtype=np.float32)
    weights = np.asarray(weights, dtype=np.float32)
    scale = np.asarray(scale, dtype=np.float32)

    wnorm = np.sqrt((weights * weights).sum(axis=1))
    sfold = (RANGE_EXTENDER * scale) / np.maximum(wnorm, EPS)
    wT = np.ascontiguousarray((weights * sfold[:, None]).T.astype(np.float16))
    wT = wT.reshape(2, 128, C)

    xh = np.ascontiguousarray(x.astype(np.float16))
    in_maps = []
    for k in range(N_CORES):
        xl = np.ascontiguousarray(xh[:, k * HL:(k + 1) * HL, :])
        in_maps.append({"x": xl.reshape(2, 128, NPIX), "wt": wT})
    return in_maps


def gather_output(results):
    outs = [res["out"].reshape(C, HL, W).astype(np.float32)
            for res in results]
    return np.concatenate(outs, axis=1)


def kernel(x, weights, scale):
    in_maps = prep_inputs(x, weights, scale)
    nc = _get_prog()
    res = run_bass_kernel_spmd(nc, in_maps, core_ids=list(range(N_CORES)))
    return gather_output(res.results)
